# revision 16
# baseline (speedup 1.0000x reference)
"""LBLHighwayBiLm Trainium2 kernel (8-core data-parallel over batch).

v2 layout: activations [D -> 2 blocks of 128 partitions, tokens on free dim],
bf16, with a uniform padded row stride of 1032 (4 halo/slack + 1024 data + 4)
for every conv source/dest so the 5-tap convs run as whole-block sliding ops.

Engine split (per cost model):
- Highway matmuls on PE (bf16, 512-col chunks into [128,1024] PSUM tiles).
- Conv: per (layer, dir, d-block) either PE (5 accumulating diagonal-lhsT
  matmuls per 512-chunk, PSUM evict via knob engine) or DVE (tensor_scalar
  4x-mode scaled copies + tensor_tensor 2x adds sliding over the whole
  padded block).
- Sigmoid eviction on ACT; relu eviction engine per-op knob (ACT/DVE/Pool).
- Highway combine t/x1 on DVE, u engine per-op knob (DVE/Pool).
Each core handles 4 batch rows; no collectives.
"""

import numpy as np
import ml_dtypes

import concourse.bacc as bacc
import concourse.tile as tile
import concourse.mybir as mybir
from concourse.bass_utils import run_bass_kernel_spmd

BF16 = mybir.dt.bfloat16
F32 = mybir.dt.float32
AOP = mybir.AluOpType
AFT = mybir.ActivationFunctionType

N_LAYERS = 2
N_HW = 2
W = 4
D = 256
B, S = 32, 1024
NCORES = 8
BLOC = B // NCORES          # 4 batch rows per core
T = BLOC * S                # 4096 tokens per core
PB = D // 128               # 2 partition blocks for D
EB = (2 * D) // 128         # 4 partition blocks for 2D
ROW = S + 2 * W             # uniform padded row stride: 1032
CH = 1024                   # token chunk = one batch row
WD = BLOC * ROW - 2 * W     # whole-block sliding op width: 4120

# --- engine assignment knobs -------------------------------------------------
# conv engine per (l, di, blk): "t" = PE diag-matmul path, "v" = DVE path.
# DVE conv lives on the fwd stream; highway emission runs bwd first so PE
# streams bwd matmuls while DVE finishes the fwd convs.
CONV_ENGINE = {
    (0, 0, 0): "v", (0, 0, 1): "v", (0, 1, 0): "t", (0, 1, 1): "t",
    (1, 0, 0): "v", (1, 0, 1): "v", (1, 1, 0): "t", (1, 1, 1): "t",
}
HW_DI_ORDER = (1, 0)   # bwd first
RELU_CYCLE = "aaaaad"  # relu PSUM-evict engine cycle: a=ACT, d=DVE (no Pool: PSUM)
CEVICT_CYCLE = "a"     # conv PSUM-evict engine cycle (PE conv path)
U_CYCLE = "g"          # u = g*t engine cycle: v=DVE, g=Pool
ENG_T = "v"            # t = x0 - r
ENG_X1 = "v"           # x1 = u + r
PSUM_BUFS = 4
SCRATCH_BUFS = 6
CONV_PREFILL = {0: 10, 1: 0}   # deferred-DVE-conv ops emitted before hwA-b
FILL_PER_SLOT = 2              # filler ops emitted per combine chunk


def _eng(nc, code):
    return {"v": nc.vector, "d": nc.vector, "g": nc.gpsimd, "p": nc.gpsimd,
            "a": nc.scalar}[code]


class _Cycle:
    def __init__(self, pattern):
        self.pattern = pattern
        self.i = 0

    def next(self):
        c = self.pattern[self.i % len(self.pattern)]
        self.i += 1
        return c


def build_bass(params):
    """params: dict of host-precomputed constant arrays / floats."""
    nc = bacc.Bacc(target_bir_lowering=False)

    x_in = nc.dram_tensor("x", [PB, 128, BLOC * ROW], BF16, kind="ExternalInput")
    out = nc.dram_tensor(
        "out", [N_LAYERS, 2, PB, 128, T], BF16, kind="ExternalOutput"
    )

    # inline constants: one packed bf16 blob (dg | pad | wt) + f32 bias
    DG_OFF = 0                      # 20 * 128 cols
    PAD_OFF = DG_OFF + 20 * 128
    WT_OFF = PAD_OFF + 2 * PB * W   # 16 * 512 cols
    CB_COLS = WT_OFF + 16 * 512
    cb_dram = nc.inline_tensor(params["cb"], name="cb")        # [128, CB_COLS] bf16
    bias_dram = nc.inline_tensor(params["bias"], name="bias")  # [128, L*2*HW*EB] f32
    fw = params["fwd_w"]  # [L, W+1] python floats
    bw = params["bwd_w"]

    relu_cycle = _Cycle(RELU_CYCLE)
    cevict_cycle = _Cycle(CEVICT_CYCLE)
    u_cycle = _Cycle(U_CYCLE)

    with tile.TileContext(nc) as tc:
        consts = tc.alloc_tile_pool(name="consts", bufs=1)
        bufs = tc.alloc_tile_pool(name="bufs", bufs=1)
        scratch = tc.alloc_tile_pool(name="scratch", bufs=SCRATCH_BUFS)
        ctmp_pool = tc.alloc_tile_pool(name="ctmp", bufs=2)
        psum = tc.alloc_tile_pool(name="psum", bufs=PSUM_BUFS, space="PSUM")

        # ---- load constants (dg+pad first: conv needs them earliest) --------
        cb_sb = consts.tile([128, CB_COLS], BF16, name="cb_sb")
        nc.gpsimd.dma_start(out=cb_sb[:, :WT_OFF], in_=cb_dram[:, :WT_OFF])

        def dg_ap(l, di, j):
            i = (l * 2 + di) * (W + 1) + j
            return cb_sb[:, DG_OFF + i * 128:DG_OFF + (i + 1) * 128]

        pad_ap_all = cb_sb[:, PAD_OFF:PAD_OFF + 2 * PB * W]

        def wt_ap(l, di, h, kb, eb):
            i = ((l * 2 + di) * N_HW + h) * PB + kb
            o = WT_OFF + i * 512 + eb * 128
            return cb_sb[:, o:o + 128]

        # ---- layer-0 padded input, per-row DMAs (conv row r starts early) ---
        xpad0 = [
            bufs.tile([128, BLOC * ROW], BF16, tag=f"xpad0_{blk}",
                      name=f"xpad0_{blk}")
            for blk in range(PB)
        ]
        for r in range(BLOC):
            for blk in range(PB):
                nc.gpsimd.dma_start(
                    out=xpad0[blk][:, r * ROW:(r + 1) * ROW],
                    in_=x_in[blk][:, r * ROW:(r + 1) * ROW],
                )
        # weights + bias after the input rows on the DMA queue
        nc.gpsimd.dma_start(out=cb_sb[:, WT_OFF:], in_=cb_dram[:, WT_OFF:])
        bias_sb = consts.tile([128, N_LAYERS * 2 * N_HW * EB], F32, name="bias_sb")
        nc.gpsimd.dma_start(out=bias_sb, in_=bias_dram[:, :])

        def bias_ap(l, di, h, eb):
            i = ((l * 2 + di) * N_HW + h) * EB + eb
            return bias_sb[:, i:i + 1]

        # ---- conv: one (l, di, blk) group -----------------------------------
        # src layout: uniform ROW-stride rows; fwd taps read base 0, bwd base W.
        # "v" groups return a list of emit-closures (deferred interleaving).
        def conv_group(l, di, blk, src_t, dst_t, taps, base):
            eng = CONV_ENGINE[(l, di, blk)]
            if eng == "v":
                ops = []
                acc = dst_t[:, 0:WD]
                ops.append(lambda acc=acc, s=src_t[:, base:base + WD],
                           w=float(taps[0]): nc.vector.tensor_scalar_mul(acc, s, w))
                for j in range(1, W + 1):
                    def scaled_add(acc=acc, s=src_t[:, base + j:base + j + WD],
                                   w=float(taps[j]), nm=f"ct{l}{di}{blk}{j}"):
                        tmp = ctmp_pool.tile([128, WD], BF16, tag="ctmp", name=nm)
                        nc.vector.tensor_scalar_mul(tmp, s, w)
                        nc.vector.tensor_tensor(acc, acc, tmp, AOP.add)
                    ops.append(scaled_add)
                return ops
            # PE: per row, 5 accumulating diag matmuls per 512-chunk
            for r in range(BLOC):
                ps = psum.tile([128, CH], F32, tag="ps",
                               name=f"cps{l}{di}{blk}{r}")
                for c0 in (0, 512):
                    o = r * ROW + base + c0
                    for j in range(W + 1):
                        nc.tensor.matmul(
                            ps[:, c0:c0 + 512],
                            lhsT=dg_ap(l, di, j),
                            rhs=src_t[:, o + j:o + j + 512],
                            start=(j == 0),
                            stop=(j == W),
                        )
                ev = cevict_cycle.next()
                dst_ap = dst_t[:, r * ROW:r * ROW + CH]
                if ev == "a":
                    nc.scalar.activation(dst_ap, ps, AFT.Copy)
                else:
                    _eng(nc, ev).tensor_copy(dst_ap, ps)
            return []

        # ---- one highway sublayer ------------------------------------------
        # x0_ap(blk, c0, n): read AP for matmul rhs / combine input
        # x1_ap(blk, tg): write AP for the combine output chunk [128, CH]
        # filler: deque of deferred DVE emit-closures, drained between chunks
        # post_chunk(blk, tg): called after each x1 write (per-tg out DMA)
        def highway(l, di, h, x0_ap, x1_ap, filler=None, post_chunk=None):
            for tg in range(T // CH):
                ps = {}
                for eb in range(EB):
                    p = psum.tile([128, CH], F32, tag="ps",
                                  name=f"ps{l}{di}{h}{eb}{tg}")
                    for kb in range(PB):
                        for half in range(CH // 512):
                            nc.tensor.matmul(
                                p[:, half * 512:(half + 1) * 512],
                                lhsT=wt_ap(l, di, h, kb, eb),
                                rhs=x0_ap(kb, tg * CH + half * 512, 512),
                                start=(kb == 0),
                                stop=(kb == PB - 1),
                            )
                    ps[eb] = p
                # nonlin = eblks [0, PB), gate = eblks [PB, 2*PB)
                for blk in range(PB):
                    gt = scratch.tile([128, CH], BF16, tag="g", name=f"g{l}{di}{h}{blk}{tg}")
                    nc.scalar.activation(
                        gt, ps[PB + blk], AFT.Sigmoid,
                        bias=bias_ap(l, di, h, PB + blk), scale=1.0,
                    )
                    rt = scratch.tile([128, CH], BF16, tag="r", name=f"r{l}{di}{h}{blk}{tg}")
                    re = relu_cycle.next()
                    if re == "a":
                        nc.scalar.activation(
                            rt, ps[blk], AFT.Relu,
                            bias=bias_ap(l, di, h, blk), scale=1.0,
                        )
                    else:
                        _eng(nc, re).tensor_scalar(
                            rt, ps[blk], bias_ap(l, di, h, blk), 0.0,
                            AOP.add, AOP.max,
                        )
                    tt = scratch.tile([128, CH], BF16, tag="t", name=f"t{l}{di}{h}{blk}{tg}")
                    x0c = x0_ap(blk, tg * CH, CH)
                    _eng(nc, ENG_T).tensor_tensor(tt, x0c, rt, AOP.subtract)
                    ut = scratch.tile([128, CH], BF16, tag="u", name=f"u{l}{di}{h}{blk}{tg}")
                    _eng(nc, u_cycle.next()).tensor_tensor(ut, gt, tt, AOP.mult)
                    _eng(nc, ENG_X1).tensor_tensor(x1_ap(blk, tg), ut, rt, AOP.add)
                    if post_chunk is not None:
                        post_chunk(blk, tg)
                    if filler:
                        for _ in range(min(FILL_PER_SLOT, len(filler))):
                            filler.popleft()()

        # ---- the network ----------------------------------------------------
        # padded access: row r data at [r*ROW + off, +CH)
        def padded_x0(tiles, off):
            def f(blk, c, n):
                r, c0 = divmod(c, CH)
                return tiles[blk][:, r * ROW + off + c0:r * ROW + off + c0 + n]
            return f

        def packed_x0(tiles):
            return lambda blk, c, n: tiles[blk][:, c:c + n]

        def padded_x1(tiles, off):
            return lambda blk, tg: tiles[blk][:, tg * ROW + off:tg * ROW + off + CH]

        def packed_x1(tiles):
            return lambda blk, tg: tiles[blk][:, tg * CH:(tg + 1) * CH]

        from collections import deque

        def emit_conv(l, src_f, src_b, f_t, filler):
            """PE groups emit immediately; DVE ('v') groups append to filler."""
            for di in (1, 0):
                for blk in range(PB):
                    taps = fw[l] if di == 0 else bw[l]
                    src = src_f if di == 0 else src_b
                    filler.extend(conv_group(
                        l, di, blk, src[blk], f_t[di][blk], taps,
                        0 if di == 0 else W))

        def alloc_f(l):
            return {
                di: [
                    bufs.tile([128, BLOC * ROW], BF16, tag=f"f{di}{blk}",
                              name=f"f{l}{di}{blk}")
                    for blk in range(PB)
                ]
                for di in range(2)
            }

        def out_dma(l, di, x1t):
            def post(blk, tg):
                nc.gpsimd.dma_start(
                    out=out[l, di, blk][:, tg * CH:(tg + 1) * CH],
                    in_=x1t[blk][:, tg * ROW + W:tg * ROW + W + CH],
                )
            return post

        # layer-0 conv + padded next-layer buffers
        f_t = alloc_f(0)
        filler = deque()
        emit_conv(0, xpad0, xpad0, f_t, filler)
        for _ in range(min(CONV_PREFILL[0], len(filler))):
            filler.popleft()()

        xpadf = [
            bufs.tile([128, BLOC * ROW], BF16, tag=f"xpf{blk}", name=f"xpf{blk}")
            for blk in range(PB)
        ]
        xpadb = [
            bufs.tile([128, BLOC * ROW], BF16, tag=f"xpb{blk}", name=f"xpb{blk}")
            for blk in range(PB)
        ]
        # halos: fwd front cols [0,W), bwd back cols [W+S, ROW)
        for blk in range(PB):
            for r in range(BLOC):
                nc.vector.tensor_copy(
                    xpadf[blk][:, r * ROW:r * ROW + W],
                    pad_ap_all[:, (0 * PB + blk) * W:(0 * PB + blk + 1) * W],
                )
                nc.vector.tensor_copy(
                    xpadb[blk][:, r * ROW + W + S:(r + 1) * ROW],
                    pad_ap_all[:, (1 * PB + blk) * W:(1 * PB + blk + 1) * W],
                )

        for l in range(N_LAYERS):
            # sublayer A: f -> xa (packed); bwd first, fwd-conv filler drains
            # into the bwd call's combine slots
            xa = {}
            for di in HW_DI_ORDER:
                if di == 0:
                    while filler:  # fwd conv must be fully emitted before mmA-f
                        filler.popleft()()
                xa[di] = [
                    bufs.tile([128, T], BF16, tag=f"xa{di}{blk}", name=f"xa{l}{di}{blk}")
                    for blk in range(PB)
                ]
                highway(l, di, 0, padded_x0(f_t[di], 0), packed_x1(xa[di]),
                        filler=filler)

            # sublayer B: xa -> padded bufs (next-layer conv src for l=0;
            # l=1 reuses the xpadf/xpadb buffers, then dead, as plain output).
            # Next layer's conv groups are emitted mid-layer: PE groups right
            # after the hwB call that produces their source, DVE groups into
            # the filler for the next layer's hwA-b slots.
            if l + 1 < N_LAYERS:
                f_t_next = alloc_f(l + 1)
            for di in HW_DI_ORDER:
                if l == 0:
                    x1t = xpadf if di == 0 else xpadb
                else:
                    tg_ = "xpf" if di == 0 else "xpb"
                    x1t = [
                        bufs.tile([128, BLOC * ROW], BF16, tag=f"{tg_}{blk}",
                                  name=f"xb{l}{di}{blk}")
                        for blk in range(PB)
                    ]
                highway(l, di, 1, packed_x0(xa[di]), padded_x1(x1t, W),
                        post_chunk=out_dma(l, di, x1t))
                if l + 1 < N_LAYERS:
                    # conv for the direction whose source is now complete
                    taps = fw[l + 1] if di == 0 else bw[l + 1]
                    for blk in range(PB):
                        filler.extend(conv_group(
                            l + 1, di, blk, x1t[blk], f_t_next[di][blk], taps,
                            0 if di == 0 else W))
            if l + 1 < N_LAYERS:
                for _ in range(min(CONV_PREFILL[l + 1], len(filler))):
                    filler.popleft()()
                f_t = f_t_next

        psum.release()
        ctmp_pool.release()
        scratch.release()
        bufs.release()
        consts.release()

    nc.finalize()
    return nc


def _prep_params(inputs):
    fwd_hw_W = np.asarray(inputs["fwd_hw_W"], np.float32)
    bwd_hw_W = np.asarray(inputs["bwd_hw_W"], np.float32)
    # lhsT layout: [l, dir, hw, kb, 128(k), 2D(e)] = W[e, k] transposed
    wt = np.empty((N_LAYERS, 2, N_HW, PB, 128, 2 * D), np.float32)
    for l in range(N_LAYERS):
        for di, Wsrc in ((0, fwd_hw_W), (1, bwd_hw_W)):
            for h in range(N_HW):
                wT = Wsrc[l, h].T  # [D, 2D]
                wt[l, di, h] = wT.reshape(PB, 128, 2 * D)
    wt = wt.astype(ml_dtypes.bfloat16)

    fwd_hw_b = np.asarray(inputs["fwd_hw_b"], np.float32)
    bwd_hw_b = np.asarray(inputs["bwd_hw_b"], np.float32)
    bias = np.empty((128, N_LAYERS * 2 * N_HW * EB), np.float32)
    for l in range(N_LAYERS):
        for di, bsrc in ((0, fwd_hw_b), (1, bwd_hw_b)):
            for h in range(N_HW):
                for eb in range(EB):
                    i = ((l * 2 + di) * N_HW + h) * EB + eb
                    bias[:, i] = bsrc[l, h, eb * 128:(eb + 1) * 128]

    # layer-1 halos: fwd front = fwd_pad[1].T, bwd back = bwd_pad[1].T
    fwd_pad = np.asarray(inputs["fwd_pad"], np.float32)
    bwd_pad = np.asarray(inputs["bwd_pad"], np.float32)
    pad1 = np.empty((128, 2 * PB * W), np.float32)
    for di, psrc in ((0, fwd_pad), (1, bwd_pad)):
        pT = psrc[1].T.reshape(PB, 128, W)  # [D, W] -> blocks
        for blk in range(PB):
            pad1[:, (di * PB + blk) * W:(di * PB + blk + 1) * W] = pT[blk]
    pad1 = pad1.astype(ml_dtypes.bfloat16)

    fwd_w = np.asarray(inputs["fwd_w"], np.float32)
    bwd_w = np.asarray(inputs["bwd_w"], np.float32)
    # diagonal tap matrices for the PE conv path
    dg = np.zeros((N_LAYERS, 2, W + 1, 128, 128), np.float32)
    for l in range(N_LAYERS):
        for di, wsrc in ((0, fwd_w), (1, bwd_w)):
            for j in range(W + 1):
                np.fill_diagonal(dg[l, di, j], wsrc[l, j])
    dg = dg.astype(ml_dtypes.bfloat16)

    # packed bf16 const blob: dg (20*128) | pad (16) | wt (16*512)
    cb = np.concatenate(
        [np.ascontiguousarray(dg.reshape(-1, 128, 128).transpose(1, 0, 2)
                              .reshape(128, -1)),
         pad1,
         np.ascontiguousarray(wt.reshape(-1, 128, 2 * D).transpose(1, 0, 2)
                              .reshape(128, -1))],
        axis=1,
    )

    return {
        "cb": np.ascontiguousarray(cb),
        "bias": np.ascontiguousarray(bias),
        "fwd_w": [[float(v) for v in row] for row in fwd_w],
        "bwd_w": [[float(v) for v in row] for row in bwd_w],
    }


def _prep_core_input(x_core, fwd_pad, bwd_pad):
    """x_core: [BLOC, S, D] f32 -> [PB, 128, BLOC*ROW] bf16 with halos."""
    xt = np.ascontiguousarray(x_core.transpose(2, 0, 1))  # [D, BLOC, S]
    blocks = xt.reshape(PB, 128, BLOC, S)
    padded = np.empty((PB, 128, BLOC, ROW), np.float32)
    padded[:, :, :, W:W + S] = blocks
    fr = fwd_pad[0].T.reshape(PB, 128, W)   # front halo (layer 0)
    bk = bwd_pad[0].T.reshape(PB, 128, W)
    padded[:, :, :, :W] = fr[:, :, None, :]
    padded[:, :, :, W + S:] = bk[:, :, None, :]
    return np.ascontiguousarray(
        padded.reshape(PB, 128, BLOC * ROW).astype(ml_dtypes.bfloat16))


_NC_CACHE = {}


def kernel(**inputs):
    params = _prep_params(inputs)
    import hashlib
    h = hashlib.sha256()
    for k in ("cb", "bias"):
        h.update(params[k].tobytes())
    h.update(repr(params["fwd_w"]).encode())
    h.update(repr(params["bwd_w"]).encode())
    key = h.hexdigest()
    if key not in _NC_CACHE:
        _NC_CACHE[key] = build_bass(params)
    nc = _NC_CACHE[key]

    x = np.asarray(inputs["inputs"], np.float32)
    fwd_pad = np.asarray(inputs["fwd_pad"], np.float32)
    bwd_pad = np.asarray(inputs["bwd_pad"], np.float32)
    in_maps = [
        {"x": _prep_core_input(x[c * BLOC:(c + 1) * BLOC], fwd_pad, bwd_pad)}
        for c in range(NCORES)
    ]
    res = run_bass_kernel_spmd(nc, in_maps, core_ids=list(range(NCORES)))

    y = np.empty((N_LAYERS, B, S, 2 * D), np.float32)
    for c in range(NCORES):
        o = np.asarray(res.results[c]["out"]).astype(np.float32)
        # [L, dir, blk, p, T] -> [L, r, s, dir*256+blk*128+p]
        o = o.reshape(N_LAYERS, 2, PB, 128, BLOC, S)
        o = o.transpose(0, 4, 5, 1, 2, 3).reshape(N_LAYERS, BLOC, S, 2 * D)
        y[:, c * BLOC:(c + 1) * BLOC] = o
    return y


# revision 17
# speedup vs baseline: 1.0376x; 1.0376x over previous
"""LBLHighwayBiLm Trainium2 kernel (8-core data-parallel over batch).

v2 layout: activations [D -> 2 blocks of 128 partitions, tokens on free dim],
bf16, with a uniform padded row stride of 1032 (4 halo/slack + 1024 data + 4)
for every conv source/dest so the 5-tap convs run as whole-block sliding ops.

Engine split (per cost model):
- Highway matmuls on PE (bf16, 512-col chunks into [128,1024] PSUM tiles).
- Conv: per (layer, dir, d-block) either PE (5 accumulating diagonal-lhsT
  matmuls per 512-chunk, PSUM evict via knob engine) or DVE (tensor_scalar
  4x-mode scaled copies + tensor_tensor 2x adds sliding over the whole
  padded block).
- Sigmoid eviction on ACT; relu eviction engine per-op knob (ACT/DVE/Pool).
- Highway combine t/x1 on DVE, u engine per-op knob (DVE/Pool).
Each core handles 4 batch rows; no collectives.
"""

import numpy as np
import ml_dtypes

import concourse.bacc as bacc
import concourse.tile as tile
import concourse.mybir as mybir
from concourse.bass_utils import run_bass_kernel_spmd

BF16 = mybir.dt.bfloat16
F32 = mybir.dt.float32
AOP = mybir.AluOpType
AFT = mybir.ActivationFunctionType

N_LAYERS = 2
N_HW = 2
W = 4
D = 256
B, S = 32, 1024
NCORES = 8
BLOC = B // NCORES          # 4 batch rows per core
T = BLOC * S                # 4096 tokens per core
PB = D // 128               # 2 partition blocks for D
EB = (2 * D) // 128         # 4 partition blocks for 2D
ROW = S + 2 * W             # uniform padded row stride: 1032
CH = 1024                   # token chunk = one batch row
WD = BLOC * ROW - 2 * W     # whole-block sliding op width: 4120

# --- engine assignment knobs -------------------------------------------------
# conv engine per (l, di, blk): "t" = PE diag-matmul path, "v" = DVE path.
# DVE conv lives on the fwd stream; highway emission runs bwd first so PE
# streams bwd matmuls while DVE finishes the fwd convs.
CONV_ENGINE = {
    (0, 0, 0): "v", (0, 0, 1): "v", (0, 1, 0): "t", (0, 1, 1): "t",
    (1, 0, 0): "v", (1, 0, 1): "v", (1, 1, 0): "t", (1, 1, 1): "t",
}
HW_DI_ORDER = (1, 0)   # bwd first
RELU_CYCLE = "aaaaad"  # relu PSUM-evict engine cycle: a=ACT, d=DVE (no Pool: PSUM)
CEVICT_CYCLE = "a"     # conv PSUM-evict engine cycle (PE conv path)
U_CYCLE = "g"          # u = g*t engine cycle: v=DVE, g=Pool
ENG_T = "v"            # t = x0 - r
ENG_X1 = "v"           # x1 = u + r
PSUM_BUFS = 4
SCRATCH_BUFS = 6
CONV_PREFILL = {0: 10, 1: 0}   # deferred-DVE-conv ops emitted before hwA-b
FILL_PER_SLOT = 2              # filler ops emitted per combine chunk


def _eng(nc, code):
    return {"v": nc.vector, "d": nc.vector, "g": nc.gpsimd, "p": nc.gpsimd,
            "a": nc.scalar}[code]


class _Cycle:
    def __init__(self, pattern):
        self.pattern = pattern
        self.i = 0

    def next(self):
        c = self.pattern[self.i % len(self.pattern)]
        self.i += 1
        return c


def build_bass(params):
    """params: dict of host-precomputed constant arrays / floats."""
    nc = bacc.Bacc(target_bir_lowering=False)

    x_in = nc.dram_tensor("x", [PB, 128, BLOC * ROW], BF16, kind="ExternalInput")
    out = nc.dram_tensor(
        "out", [N_LAYERS, 2, PB, 128, T], BF16, kind="ExternalOutput"
    )

    # inline constants: one packed bf16 blob (dg | pad | wt) + f32 bias
    DG_OFF = 0                      # 20 * 128 cols
    PAD_OFF = DG_OFF + 20 * 128
    WT_OFF = PAD_OFF + 2 * PB * W   # 16 * 512 cols
    CB_COLS = WT_OFF + 16 * 512
    cb_dram = nc.inline_tensor(params["cb"], name="cb")        # [128, CB_COLS] bf16
    bias_dram = nc.inline_tensor(params["bias"], name="bias")  # [128, L*2*HW*EB] f32
    fw = params["fwd_w"]  # [L, W+1] python floats
    bw = params["bwd_w"]

    relu_cycle = _Cycle(RELU_CYCLE)
    cevict_cycle = _Cycle(CEVICT_CYCLE)
    u_cycle = _Cycle(U_CYCLE)

    with tile.TileContext(nc) as tc:
        consts = tc.alloc_tile_pool(name="consts", bufs=1)
        bufs = tc.alloc_tile_pool(name="bufs", bufs=1)
        scratch = tc.alloc_tile_pool(name="scratch", bufs=SCRATCH_BUFS)
        ctmp_pool = tc.alloc_tile_pool(name="ctmp", bufs=2)
        psum = tc.alloc_tile_pool(name="psum", bufs=PSUM_BUFS, space="PSUM")

        # ---- load constants (dg+pad first: conv needs them earliest) --------
        cb_sb = consts.tile([128, CB_COLS], BF16, name="cb_sb")
        nc.sync.dma_start(out=cb_sb[:, :WT_OFF], in_=cb_dram[:, :WT_OFF])

        def dg_ap(l, di, j):
            i = (l * 2 + di) * (W + 1) + j
            return cb_sb[:, DG_OFF + i * 128:DG_OFF + (i + 1) * 128]

        pad_ap_all = cb_sb[:, PAD_OFF:PAD_OFF + 2 * PB * W]

        def wt_ap(l, di, h, kb, eb):
            i = ((l * 2 + di) * N_HW + h) * PB + kb
            o = WT_OFF + i * 512 + eb * 128
            return cb_sb[:, o:o + 128]

        # ---- layer-0 padded input, per-row DMAs (conv row r starts early) ---
        xpad0 = [
            bufs.tile([128, BLOC * ROW], BF16, tag=f"xpad0_{blk}",
                      name=f"xpad0_{blk}")
            for blk in range(PB)
        ]
        for r in range(BLOC):
            for blk in range(PB):
                nc.sync.dma_start(
                    out=xpad0[blk][:, r * ROW:(r + 1) * ROW],
                    in_=x_in[blk][:, r * ROW:(r + 1) * ROW],
                )
        # weights + bias after the input rows on the DMA queue
        nc.sync.dma_start(out=cb_sb[:, WT_OFF:], in_=cb_dram[:, WT_OFF:])
        bias_sb = consts.tile([128, N_LAYERS * 2 * N_HW * EB], F32, name="bias_sb")
        nc.sync.dma_start(out=bias_sb, in_=bias_dram[:, :])

        def bias_ap(l, di, h, eb):
            i = ((l * 2 + di) * N_HW + h) * EB + eb
            return bias_sb[:, i:i + 1]

        # ---- conv: one (l, di, blk) group -----------------------------------
        # src layout: uniform ROW-stride rows; fwd taps read base 0, bwd base W.
        # "v" groups return a list of emit-closures (deferred interleaving).
        def conv_group(l, di, blk, src_t, dst_t, taps, base):
            eng = CONV_ENGINE[(l, di, blk)]
            if eng == "v":
                ops = []
                acc = dst_t[:, 0:WD]
                ops.append(lambda acc=acc, s=src_t[:, base:base + WD],
                           w=float(taps[0]): nc.vector.tensor_scalar_mul(acc, s, w))
                for j in range(1, W + 1):
                    def scaled_add(acc=acc, s=src_t[:, base + j:base + j + WD],
                                   w=float(taps[j]), nm=f"ct{l}{di}{blk}{j}"):
                        tmp = ctmp_pool.tile([128, WD], BF16, tag="ctmp", name=nm)
                        nc.vector.tensor_scalar_mul(tmp, s, w)
                        nc.vector.tensor_tensor(acc, acc, tmp, AOP.add)
                    ops.append(scaled_add)
                return ops
            # PE: per row, 5 accumulating diag matmuls per 512-chunk
            for r in range(BLOC):
                ps = psum.tile([128, CH], F32, tag="ps",
                               name=f"cps{l}{di}{blk}{r}")
                for c0 in (0, 512):
                    o = r * ROW + base + c0
                    for j in range(W + 1):
                        nc.tensor.matmul(
                            ps[:, c0:c0 + 512],
                            lhsT=dg_ap(l, di, j),
                            rhs=src_t[:, o + j:o + j + 512],
                            start=(j == 0),
                            stop=(j == W),
                        )
                ev = cevict_cycle.next()
                dst_ap = dst_t[:, r * ROW:r * ROW + CH]
                if ev == "a":
                    nc.scalar.activation(dst_ap, ps, AFT.Copy)
                else:
                    _eng(nc, ev).tensor_copy(dst_ap, ps)
            return []

        # ---- one highway sublayer ------------------------------------------
        # x0_ap(blk, c0, n): read AP for matmul rhs / combine input
        # x1_ap(blk, tg): write AP for the combine output chunk [128, CH]
        # filler: deque of deferred DVE emit-closures, drained between chunks
        # post_chunk(blk, tg): called after each x1 write (per-tg out DMA)
        def highway(l, di, h, x0_ap, x1_ap, filler=None, post_chunk=None):
            for tg in range(T // CH):
                ps = {}
                for eb in range(EB):
                    p = psum.tile([128, CH], F32, tag="ps",
                                  name=f"ps{l}{di}{h}{eb}{tg}")
                    for kb in range(PB):
                        for half in range(CH // 512):
                            nc.tensor.matmul(
                                p[:, half * 512:(half + 1) * 512],
                                lhsT=wt_ap(l, di, h, kb, eb),
                                rhs=x0_ap(kb, tg * CH + half * 512, 512),
                                start=(kb == 0),
                                stop=(kb == PB - 1),
                            )
                    ps[eb] = p
                # nonlin = eblks [0, PB), gate = eblks [PB, 2*PB)
                for blk in range(PB):
                    gt = scratch.tile([128, CH], BF16, tag="g", name=f"g{l}{di}{h}{blk}{tg}")
                    nc.scalar.activation(
                        gt, ps[PB + blk], AFT.Sigmoid,
                        bias=bias_ap(l, di, h, PB + blk), scale=1.0,
                    )
                    rt = scratch.tile([128, CH], BF16, tag="r", name=f"r{l}{di}{h}{blk}{tg}")
                    re = relu_cycle.next()
                    if re == "a":
                        nc.scalar.activation(
                            rt, ps[blk], AFT.Relu,
                            bias=bias_ap(l, di, h, blk), scale=1.0,
                        )
                    else:
                        _eng(nc, re).tensor_scalar(
                            rt, ps[blk], bias_ap(l, di, h, blk), 0.0,
                            AOP.add, AOP.max,
                        )
                    tt = scratch.tile([128, CH], BF16, tag="t", name=f"t{l}{di}{h}{blk}{tg}")
                    x0c = x0_ap(blk, tg * CH, CH)
                    _eng(nc, ENG_T).tensor_tensor(tt, x0c, rt, AOP.subtract)
                    ut = scratch.tile([128, CH], BF16, tag="u", name=f"u{l}{di}{h}{blk}{tg}")
                    _eng(nc, u_cycle.next()).tensor_tensor(ut, gt, tt, AOP.mult)
                    _eng(nc, ENG_X1).tensor_tensor(x1_ap(blk, tg), ut, rt, AOP.add)
                    if post_chunk is not None:
                        post_chunk(blk, tg)
                    if filler:
                        for _ in range(min(FILL_PER_SLOT, len(filler))):
                            filler.popleft()()

        # ---- the network ----------------------------------------------------
        # padded access: row r data at [r*ROW + off, +CH)
        def padded_x0(tiles, off):
            def f(blk, c, n):
                r, c0 = divmod(c, CH)
                return tiles[blk][:, r * ROW + off + c0:r * ROW + off + c0 + n]
            return f

        def packed_x0(tiles):
            return lambda blk, c, n: tiles[blk][:, c:c + n]

        def padded_x1(tiles, off):
            return lambda blk, tg: tiles[blk][:, tg * ROW + off:tg * ROW + off + CH]

        def packed_x1(tiles):
            return lambda blk, tg: tiles[blk][:, tg * CH:(tg + 1) * CH]

        from collections import deque

        def emit_conv(l, src_f, src_b, f_t, filler):
            """PE groups emit immediately; DVE ('v') groups append to filler."""
            for di in (1, 0):
                for blk in range(PB):
                    taps = fw[l] if di == 0 else bw[l]
                    src = src_f if di == 0 else src_b
                    filler.extend(conv_group(
                        l, di, blk, src[blk], f_t[di][blk], taps,
                        0 if di == 0 else W))

        def alloc_f(l):
            return {
                di: [
                    bufs.tile([128, BLOC * ROW], BF16, tag=f"f{di}{blk}",
                              name=f"f{l}{di}{blk}")
                    for blk in range(PB)
                ]
                for di in range(2)
            }

        def out_dma(l, di, x1t):
            def post(blk, tg):
                nc.sync.dma_start(
                    out=out[l, di, blk][:, tg * CH:(tg + 1) * CH],
                    in_=x1t[blk][:, tg * ROW + W:tg * ROW + W + CH],
                )
            return post

        # layer-0 conv + padded next-layer buffers
        f_t = alloc_f(0)
        filler = deque()
        emit_conv(0, xpad0, xpad0, f_t, filler)
        for _ in range(min(CONV_PREFILL[0], len(filler))):
            filler.popleft()()

        xpadf = [
            bufs.tile([128, BLOC * ROW], BF16, tag=f"xpf{blk}", name=f"xpf{blk}")
            for blk in range(PB)
        ]
        xpadb = [
            bufs.tile([128, BLOC * ROW], BF16, tag=f"xpb{blk}", name=f"xpb{blk}")
            for blk in range(PB)
        ]
        # halos: fwd front cols [0,W), bwd back cols [W+S, ROW)
        for blk in range(PB):
            for r in range(BLOC):
                nc.vector.tensor_copy(
                    xpadf[blk][:, r * ROW:r * ROW + W],
                    pad_ap_all[:, (0 * PB + blk) * W:(0 * PB + blk + 1) * W],
                )
                nc.vector.tensor_copy(
                    xpadb[blk][:, r * ROW + W + S:(r + 1) * ROW],
                    pad_ap_all[:, (1 * PB + blk) * W:(1 * PB + blk + 1) * W],
                )

        for l in range(N_LAYERS):
            # sublayer A: f -> xa (packed); bwd first, fwd-conv filler drains
            # into the bwd call's combine slots
            xa = {}
            for di in HW_DI_ORDER:
                if di == 0:
                    while filler:  # fwd conv must be fully emitted before mmA-f
                        filler.popleft()()
                xa[di] = [
                    bufs.tile([128, T], BF16, tag=f"xa{di}{blk}", name=f"xa{l}{di}{blk}")
                    for blk in range(PB)
                ]
                highway(l, di, 0, padded_x0(f_t[di], 0), packed_x1(xa[di]),
                        filler=filler)

            # sublayer B: xa -> padded bufs (next-layer conv src for l=0;
            # l=1 reuses the xpadf/xpadb buffers, then dead, as plain output).
            # Next layer's conv groups are emitted mid-layer: PE groups right
            # after the hwB call that produces their source, DVE groups into
            # the filler for the next layer's hwA-b slots.
            if l + 1 < N_LAYERS:
                f_t_next = alloc_f(l + 1)
            for di in HW_DI_ORDER:
                if l == 0:
                    x1t = xpadf if di == 0 else xpadb
                else:
                    tg_ = "xpf" if di == 0 else "xpb"
                    x1t = [
                        bufs.tile([128, BLOC * ROW], BF16, tag=f"{tg_}{blk}",
                                  name=f"xb{l}{di}{blk}")
                        for blk in range(PB)
                    ]
                highway(l, di, 1, packed_x0(xa[di]), padded_x1(x1t, W),
                        post_chunk=out_dma(l, di, x1t))
                if l + 1 < N_LAYERS:
                    # conv for the direction whose source is now complete
                    taps = fw[l + 1] if di == 0 else bw[l + 1]
                    for blk in range(PB):
                        filler.extend(conv_group(
                            l + 1, di, blk, x1t[blk], f_t_next[di][blk], taps,
                            0 if di == 0 else W))
            if l + 1 < N_LAYERS:
                for _ in range(min(CONV_PREFILL[l + 1], len(filler))):
                    filler.popleft()()
                f_t = f_t_next

        psum.release()
        ctmp_pool.release()
        scratch.release()
        bufs.release()
        consts.release()

    nc.finalize()
    return nc


def _prep_params(inputs):
    fwd_hw_W = np.asarray(inputs["fwd_hw_W"], np.float32)
    bwd_hw_W = np.asarray(inputs["bwd_hw_W"], np.float32)
    # lhsT layout: [l, dir, hw, kb, 128(k), 2D(e)] = W[e, k] transposed
    wt = np.empty((N_LAYERS, 2, N_HW, PB, 128, 2 * D), np.float32)
    for l in range(N_LAYERS):
        for di, Wsrc in ((0, fwd_hw_W), (1, bwd_hw_W)):
            for h in range(N_HW):
                wT = Wsrc[l, h].T  # [D, 2D]
                wt[l, di, h] = wT.reshape(PB, 128, 2 * D)
    wt = wt.astype(ml_dtypes.bfloat16)

    fwd_hw_b = np.asarray(inputs["fwd_hw_b"], np.float32)
    bwd_hw_b = np.asarray(inputs["bwd_hw_b"], np.float32)
    bias = np.empty((128, N_LAYERS * 2 * N_HW * EB), np.float32)
    for l in range(N_LAYERS):
        for di, bsrc in ((0, fwd_hw_b), (1, bwd_hw_b)):
            for h in range(N_HW):
                for eb in range(EB):
                    i = ((l * 2 + di) * N_HW + h) * EB + eb
                    bias[:, i] = bsrc[l, h, eb * 128:(eb + 1) * 128]

    # layer-1 halos: fwd front = fwd_pad[1].T, bwd back = bwd_pad[1].T
    fwd_pad = np.asarray(inputs["fwd_pad"], np.float32)
    bwd_pad = np.asarray(inputs["bwd_pad"], np.float32)
    pad1 = np.empty((128, 2 * PB * W), np.float32)
    for di, psrc in ((0, fwd_pad), (1, bwd_pad)):
        pT = psrc[1].T.reshape(PB, 128, W)  # [D, W] -> blocks
        for blk in range(PB):
            pad1[:, (di * PB + blk) * W:(di * PB + blk + 1) * W] = pT[blk]
    pad1 = pad1.astype(ml_dtypes.bfloat16)

    fwd_w = np.asarray(inputs["fwd_w"], np.float32)
    bwd_w = np.asarray(inputs["bwd_w"], np.float32)
    # diagonal tap matrices for the PE conv path
    dg = np.zeros((N_LAYERS, 2, W + 1, 128, 128), np.float32)
    for l in range(N_LAYERS):
        for di, wsrc in ((0, fwd_w), (1, bwd_w)):
            for j in range(W + 1):
                np.fill_diagonal(dg[l, di, j], wsrc[l, j])
    dg = dg.astype(ml_dtypes.bfloat16)

    # packed bf16 const blob: dg (20*128) | pad (16) | wt (16*512)
    cb = np.concatenate(
        [np.ascontiguousarray(dg.reshape(-1, 128, 128).transpose(1, 0, 2)
                              .reshape(128, -1)),
         pad1,
         np.ascontiguousarray(wt.reshape(-1, 128, 2 * D).transpose(1, 0, 2)
                              .reshape(128, -1))],
        axis=1,
    )

    return {
        "cb": np.ascontiguousarray(cb),
        "bias": np.ascontiguousarray(bias),
        "fwd_w": [[float(v) for v in row] for row in fwd_w],
        "bwd_w": [[float(v) for v in row] for row in bwd_w],
    }


def _prep_core_input(x_core, fwd_pad, bwd_pad):
    """x_core: [BLOC, S, D] f32 -> [PB, 128, BLOC*ROW] bf16 with halos."""
    xt = np.ascontiguousarray(x_core.transpose(2, 0, 1))  # [D, BLOC, S]
    blocks = xt.reshape(PB, 128, BLOC, S)
    padded = np.empty((PB, 128, BLOC, ROW), np.float32)
    padded[:, :, :, W:W + S] = blocks
    fr = fwd_pad[0].T.reshape(PB, 128, W)   # front halo (layer 0)
    bk = bwd_pad[0].T.reshape(PB, 128, W)
    padded[:, :, :, :W] = fr[:, :, None, :]
    padded[:, :, :, W + S:] = bk[:, :, None, :]
    return np.ascontiguousarray(
        padded.reshape(PB, 128, BLOC * ROW).astype(ml_dtypes.bfloat16))


_NC_CACHE = {}


def kernel(**inputs):
    params = _prep_params(inputs)
    import hashlib
    h = hashlib.sha256()
    for k in ("cb", "bias"):
        h.update(params[k].tobytes())
    h.update(repr(params["fwd_w"]).encode())
    h.update(repr(params["bwd_w"]).encode())
    key = h.hexdigest()
    if key not in _NC_CACHE:
        _NC_CACHE[key] = build_bass(params)
    nc = _NC_CACHE[key]

    x = np.asarray(inputs["inputs"], np.float32)
    fwd_pad = np.asarray(inputs["fwd_pad"], np.float32)
    bwd_pad = np.asarray(inputs["bwd_pad"], np.float32)
    in_maps = [
        {"x": _prep_core_input(x[c * BLOC:(c + 1) * BLOC], fwd_pad, bwd_pad)}
        for c in range(NCORES)
    ]
    res = run_bass_kernel_spmd(nc, in_maps, core_ids=list(range(NCORES)))

    y = np.empty((N_LAYERS, B, S, 2 * D), np.float32)
    for c in range(NCORES):
        o = np.asarray(res.results[c]["out"]).astype(np.float32)
        # [L, dir, blk, p, T] -> [L, r, s, dir*256+blk*128+p]
        o = o.reshape(N_LAYERS, 2, PB, 128, BLOC, S)
        o = o.transpose(0, 4, 5, 1, 2, 3).reshape(N_LAYERS, BLOC, S, 2 * D)
        y[:, c * BLOC:(c + 1) * BLOC] = o
    return y
